# revision 34
# baseline (speedup 1.0000x reference)
#!/usr/bin/env python
"""Multi-head attention (nn_MultiHeadAttention) Trainium2 Bass kernel, v2.

Problem: B=8, S=1024, n_hidden=1024, 16 heads x 64 dim. V projection == K
projection (reference quirk). Output = softmax(mask + QK^T/8) @ K @ Wo + bo.

Strategy: batch-parallel across the 8 NeuronCores (core b handles batch b,
weights replicated, zero collectives). Per core, a software-pipelined loop
over the 8 hidden tiles t (= head pairs 2t, 2t+1) keeps PE, ACT and DVE all
busy:

  iteration t emits, interleaved per key-chunk kc:
    logits^T(t, kc)   [128k, 1024q] = (K^T_t)^T-contract Q^T_t   (PE, fp32r)
    E(t, kc)          = exp(logits^T/8 + mask_bias) -> bf16       (ACT)
    att(t-1, qc=kc)   [128q, 2x65]  = E^T-contract V_aug (bf16 PE; column
                      64 of each head block = softmax denominator via the
                      ones column carried in V)
    normalize         DVE reciprocal + per-partition tensor_scalar_mul
    att^T(t-1)        PE transposes of the normalized [128q, 128d] blocks
                      (head pair packed on partitions) -> attT tile layout
    proj(t+1)         Q^T/K^T m-tile projections + V transposes (PE + DVE)

  epilogue: out[q, m] = attT^T-contract Wo (bf16) + bo  (direct DRAM layout)

The softmax skips the max-subtraction: logits are O(6), exp stays in fp32
range, masked keys produce exp(-1e9) == 0 exactly. Scores/V/att/Wo run in
bf16 (errors ~0.4%, far inside the 2e-2 gate); the x/Wq/Wk/logits path stays
fp32r.
"""
import sys
import os

sys.path.insert(0, "/opt/trn_rl_repo")
os.environ.setdefault("JAX_COMPILATION_CACHE_DIR", "/tmp/jax_comp_cache")

import numpy as np

B, S, H, NH, DH = 8, 1024, 1024, 16, 64
NT = H // 128   # 8 partition tiles of hidden (= head pairs)
NCH = S // 128  # 8 key chunks
NQ = S // 512   # 2 query 512-tiles
DV = DH + 1     # V block width (ones column at 64)

_cache = {}


def _build_nc(repeat=1):
    import concourse.bacc as bacc
    import concourse.tile as tile
    from concourse import mybir
    from contextlib import ExitStack

    F32 = mybir.dt.float32
    F32R = mybir.dt.float32r
    BF16 = mybir.dt.bfloat16

    nc = bacc.Bacc("TRN2", target_bir_lowering=False, debug=False)

    x_d = nc.dram_tensor("x", [S, H], BF16, kind="ExternalInput").ap()
    maskf_d = nc.dram_tensor("maskf", [128, NCH], F32, kind="ExternalInput").ap()
    wq_d = nc.dram_tensor("wq", [H, H], BF16, kind="ExternalInput").ap()  # pre-tiled [m*128+p, k*128+mm]
    wk_d = nc.dram_tensor("wk", [H, H], BF16, kind="ExternalInput").ap()  # pre-tiled
    wo_d = nc.dram_tensor("wo", [H, H], BF16, kind="ExternalInput").ap()
    bqr_d = nc.dram_tensor("bqr", [128, NT], F32, kind="ExternalInput").ap()
    bkr_d = nc.dram_tensor("bkr", [128, NT], F32, kind="ExternalInput").ap()
    bo_d = nc.dram_tensor("bo_bc", [128, H], F32, kind="ExternalInput").ap()
    id_d = nc.dram_tensor("ident", [128, 128], F32R, kind="ExternalInput").ap()
    idb_d = nc.dram_tensor("identb", [128, 128], BF16, kind="ExternalInput").ap()
    out_d = nc.dram_tensor("out", [S, H], F32, kind="ExternalOutput").ap()

    with tile.TileContext(nc) as tc, ExitStack() as top:
        misc = top.enter_context(tc.tile_pool(name="misc", bufs=1))
        maskf = misc.tile([128, NCH], F32)
        bqr = misc.tile([128, NT], F32)
        bkr = misc.tile([128, NT], F32)
        bo_bc = misc.tile([128, H], F32)
        ident = misc.tile([128, 128], F32R)
        identb = misc.tile([128, 128], BF16)
        nc.sync.dma_start(ident[:], id_d)

        for _rep in range(repeat):
            _emit_body(
                nc, tc, tile, mybir, ExitStack,
                x_d, wq_d, wk_d, wo_d, out_d,
                maskf, bqr, bkr, bo_bc, ident, identb,
                maskf_d, bqr_d, bkr_d, bo_d, idb_d,
            )

    nc.compile()
    return nc


def _emit_body(nc, tc, tile, mybir, ExitStack,
               x_d, wq_d, wk_d, wo_d, out_d,
               maskf, bqr, bkr, bo_bc, ident, identb,
               maskf_d, bqr_d, bkr_d, bo_d, idb_d):
    F32 = mybir.dt.float32
    F32R = mybir.dt.float32r
    BF16 = mybir.dt.bfloat16
    AF = mybir.ActivationFunctionType

    with ExitStack() as body:
        xT_p = body.enter_context(tc.tile_pool(name="xT", bufs=1))
        xT = xT_p.tile([128, NT * S], BF16)
        wst_p = body.enter_context(tc.tile_pool(name="wst", bufs=4))

        Wq_sb = {}    # t -> staged weight tile
        Wk_sb = {}

        def dma_w(t):
            Wq_sb[t] = wst_p.tile([128, H], BF16, tag="w", name=f"wq_{t}")
            Wk_sb[t] = wst_p.tile([128, H], BF16, tag="w", name=f"wk_{t}")
            nc.sync.dma_start(Wq_sb[t][:], wq_d[t * 128 : (t + 1) * 128, :])
            nc.sync.dma_start(Wk_sb[t][:], wk_d[t * 128 : (t + 1) * 128, :])

        # ---- Phase A: load x, transpose to x^T ---------------------------
        # x chunks alternate between the SP and ACT HWDGE queues so two DMA
        # engines stream in parallel; ident went out first (transposes need
        # it), all other constants queue up behind the x chunks.
        with tc.tile_pool(name="xs", bufs=1) as xs_p, \
             tc.tile_pool(name="tp", bufs=4, space="PSUM") as tp_p:
            xs = xs_p.tile([128, NCH * H], BF16)
            for sc in range(NCH):
                eng = nc.sync if sc % 2 == 0 else nc.scalar
                eng.dma_start(
                    xs[:, sc * H : (sc + 1) * H],
                    x_d[sc * 128 : (sc + 1) * 128, :],
                )
            dma_w(0)
            nc.scalar.dma_start(maskf[:], maskf_d)
            nc.scalar.dma_start(identb[:], idb_d)
            dma_w(1)
            nc.sync.dma_start(bqr[:], bqr_d)
            nc.sync.dma_start(bkr[:], bkr_d)
            nc.sync.dma_start(bo_bc[:], bo_d)
            for g in range(2):
                for hc in range(NT):
                    pt = tp_p.tile([128, 512], BF16, tag="tp", name="pt")
                    for j in range(4):
                        sc = g * 4 + j
                        nc.tensor.transpose(
                            pt[:, 128 * j : 128 * (j + 1)],
                            xs[:, sc * H + hc * 128 : sc * H + (hc + 1) * 128],
                            identb[:],
                        )
                    xt_dst = xT[:, hc * S + g * 512 : hc * S + (g + 1) * 512]
                    if hc % 2 == 0:
                        nc.vector.tensor_copy(xt_dst, pt[:])
                    else:
                        nc.scalar.activation(xt_dst, pt[:], AF.Identity, bias=0.0)

        # ---- persistent/per-tile state -----------------------------------
        QT_p = body.enter_context(tc.tile_pool(name="QTp", bufs=3))
        KT_p = body.enter_context(tc.tile_pool(name="KTp", bufs=3))
        V_p = body.enter_context(tc.tile_pool(name="Vp", bufs=1))
        E_p = body.enter_context(tc.tile_pool(name="Ep", bufs=32))
        attT_p = body.enter_context(tc.tile_pool(name="attTp", bufs=1))
        asb_p = body.enter_context(tc.tile_pool(name="asbp", bufs=2))
        rc_p = body.enter_context(tc.tile_pool(name="rcp", bufs=2))

        V = V_p.tile([128, NH * NCH * DV], BF16)
        attT = attT_p.tile([128, NT * S], BF16)
        V_blocks = V[:].rearrange("p (g o) -> p g o", o=DV)
        nc.vector.memset(V_blocks[:, :, DH : DH + 1], 1.0)

        QT = {}       # t -> [128, S] tile (head pair 2t,2t+1 on partitions)
        KT = {}
        E_t = {}      # (t, h2, kc) -> E tile
        asb = {}      # t -> normalized att sbuf tile [128, S] bf16
        rct = {}      # t -> reciprocal tile [128, 16]

        def proj_piece(t, piece, pp_p):
            # pieces 0..3: Q/K projections by 512-chunk; 4,5: V transposes
            if piece < 4:
                is_q = piece < 2
                n = piece % 2
                w_m = Wq_sb[t] if is_q else Wk_sb[t]
                brow = bqr if is_q else bkr
                dct, pool, tg = (QT, QT_p, "qt") if is_q else (KT, KT_p, "kt")
                if n == 0:
                    dct[t] = pool.tile([128, S], BF16, tag=tg, name=f"{tg}_{t}")
                dst = dct[t]
                pp = pp_p.tile([128, 512], F32, tag="pp", name="pp")
                for k in range(NT):
                    nc.tensor.matmul(
                        pp[:],
                        w_m[:, k * 128 : (k + 1) * 128],
                        xT[:, k * S + n * 512 : k * S + (n + 1) * 512],
                        start=(k == 0),
                        stop=(k == NT - 1),
                    )
                nc.vector.tensor_scalar_add(
                    dst[:, n * 512 : (n + 1) * 512], pp[:], brow[:, t : t + 1]
                )
            else:
                h2 = piece - 4
                h = 2 * t + h2
                pv = pp_p.tile([128, 512], BF16, tag="pp", name="pv")
                for c in range(NCH):
                    nc.tensor.transpose(
                        pv[:, c * DH : (c + 1) * DH],
                        KT[t][64 * h2 : 64 * h2 + 64, c * 128 : (c + 1) * 128],
                        identb[64 * h2 : 64 * h2 + 64, 64 * h2 : 64 * h2 + 64],
                    )
                nc.vector.tensor_copy(
                    V_blocks[:, h * NCH : (h + 1) * NCH, 0:DH],
                    pv[:].rearrange("p (c d) -> p c d", d=DH),
                )

        def lg_exp(t, kc, lg_p):
            for h2 in range(2):
                lg = lg_p.tile([128, S], F32, tag="lg", name="lg")
                for n in range(NQ):
                    nc.tensor.matmul(
                        lg[:, n * 512 : (n + 1) * 512],
                        KT[t][64 * h2 : 64 * h2 + 64, kc * 128 : (kc + 1) * 128],
                        QT[t][64 * h2 : 64 * h2 + 64, n * 512 : (n + 1) * 512],
                        start=True,
                        stop=True,
                    )
                E = E_p.tile([128, S], BF16, tag="E", name="E")
                nc.scalar.activation(
                    E[:], lg[:], AF.Exp, bias=maskf[:, kc : kc + 1], scale=0.125
                )
                E_t[(t, h2, kc)] = E

        def att_qc(t, qc, att_p, lg_p=None):
            if qc == 0:
                asb[t] = asb_p.tile([128, S], BF16, tag="asb", name=f"asb_{t}")
                rct[t] = rc_p.tile([128, 16], F32, tag="rc", name=f"rc_{t}")
            if lg_p is not None and qc % 2 == 1:
                # final tile: lg pool is idle, borrow it for ring depth 4
                attb = lg_p.tile([128, 2 * DV], F32, tag="lg", name="attb")
            else:
                attb = att_p.tile([128, 2 * DV], F32, tag="att", name="attb")
            for h2 in range(2):
                h = 2 * t + h2
                for kc in range(NCH):
                    nc.tensor.matmul(
                        attb[:, DV * h2 : DV * h2 + DV],
                        E_t[(t, h2, kc)][:, qc * 128 : (qc + 1) * 128],
                        V[:, (h * NCH + kc) * DV : (h * NCH + kc + 1) * DV],
                        start=(kc == 0),
                        stop=(kc == NCH - 1),
                    )
            for h2 in range(2):
                nc.vector.reciprocal(
                    rct[t][:, 2 * qc + h2 : 2 * qc + h2 + 1],
                    attb[:, DV * h2 + DH : DV * h2 + DH + 1],
                )
            for h2 in range(2):
                nc.vector.tensor_scalar_mul(
                    asb[t][:, qc * 128 + 64 * h2 : qc * 128 + 64 * h2 + 64],
                    attb[:, DV * h2 : DV * h2 + DH],
                    rct[t][:, 2 * qc + h2 : 2 * qc + h2 + 1],
                )

        def attT_tp(t, g, pp_p):
            pt = pp_p.tile([128, 512], BF16, tag="pp", name="ptt")
            for j in range(4):
                qc = 4 * g + j
                nc.tensor.transpose(
                    pt[:, j * 128 : (j + 1) * 128],
                    asb[t][:, qc * 128 : (qc + 1) * 128],
                    identb[:],
                )
            nc.vector.tensor_copy(
                attT[:, t * S + g * 512 : t * S + (g + 1) * 512], pt[:]
            )

        # ---- main software-pipelined loop --------------------------------
        wo_sb = None
        with tc.tile_pool(name="wo", bufs=1) as wo_p:
            with tc.tile_pool(name="pp", bufs=2, space="PSUM") as pp_p, \
                 tc.tile_pool(name="lg", bufs=2, space="PSUM") as lg_p, \
                 tc.tile_pool(name="att", bufs=2, space="PSUM") as att_p:
                for piece in range(6):
                    proj_piece(0, piece, pp_p)
                for t in range(NT + 1):
                    for kc in range(NCH):
                        if t < NT:
                            lg_exp(t, kc, lg_p)
                        if t >= 1:
                            att_qc(t - 1, kc, att_p, lg_p if t == NT else None)
                            if kc == 3:
                                attT_tp(t - 1, 0, pp_p)
                            elif kc == 7:
                                attT_tp(t - 1, 1, pp_p)
                        if t + 1 < NT and 1 <= kc <= 6:
                            if kc == 1 and t + 2 < NT:
                                dma_w(t + 2)
                            proj_piece(t + 1, kc - 1, pp_p)
                        if t == NT - 1 and kc == 0:
                            # prefetch Wo while the last attention tiles run
                            wo_sb = wo_p.tile([128, NT * H], BF16)
                            for c in range(NT):
                                nc.sync.dma_start(
                                    wo_sb[:, c * H : (c + 1) * H],
                                    wo_d[c * 128 : (c + 1) * 128, :],
                                )
                    if t >= 1:
                        for h2 in range(2):
                            for kc in range(NCH):
                                E_t.pop((t - 1, h2, kc), None)

                # ---- Phase E: output projection. Reuses the pp/asb rings
                # (no new pools -> no drain barrier before the first matmul).
                for qt in range(NT):
                    for mt in range(NQ):
                        po = pp_p.tile([128, 512], F32, tag="pp", name="po")
                        for c in range(NT):
                            nc.tensor.matmul(
                                po[:],
                                attT[:, c * S + qt * 128 : c * S + (qt + 1) * 128],
                                wo_sb[:, c * H + mt * 512 : c * H + (mt + 1) * 512],
                                start=(c == 0),
                                stop=(c == NT - 1),
                            )
                        # weight-staging ring is retired by now; borrow it so
                        # four out-tiles can be in flight over the DMA queues
                        ob = wst_p.tile([128, 512], F32, tag="w", name="ob")
                        nc.vector.tensor_add(
                            ob[:], po[:], bo_bc[:, mt * 512 : (mt + 1) * 512]
                        )
                        eng = nc.sync if (qt + mt) % 2 == 0 else nc.scalar
                        eng.dma_start(
                            out_d[qt * 128 : (qt + 1) * 128, mt * 512 : (mt + 1) * 512],
                            ob[:],
                        )


def _host_inputs(inputs):
    """Host-side prep: per-core input dicts (core b <- batch b)."""
    import ml_dtypes

    x = np.asarray(inputs["x"], dtype=np.float32)
    mask = np.asarray(inputs["padding_mask"])

    def _pretile(w):
        # w[k*128+p, m*128+mm] -> out[m*128+p, k*128+mm]
        w = np.asarray(w, dtype=np.float32).reshape(NT, 128, NT, 128)
        return np.ascontiguousarray(w.transpose(2, 1, 0, 3).reshape(H, H))

    wq = _pretile(inputs["Wq"]).astype(ml_dtypes.bfloat16)
    wk = _pretile(inputs["Wk"]).astype(ml_dtypes.bfloat16)
    wo = np.ascontiguousarray(
        np.asarray(inputs["Wo"], dtype=np.float32).astype(ml_dtypes.bfloat16)
    )
    bq = np.asarray(inputs["bq"], dtype=np.float32)
    bk = np.asarray(inputs["bk"], dtype=np.float32)
    bo = np.asarray(inputs["bo"], dtype=np.float32)

    bqr = np.ascontiguousarray(bq.reshape(NT, 128).T)
    bkr = np.ascontiguousarray(bk.reshape(NT, 128).T)
    bo_bc = np.ascontiguousarray(np.tile(bo[None, :], (128, 1)))
    ident = np.eye(128, dtype=np.float32)
    identb = np.eye(128, dtype=np.float32).astype(ml_dtypes.bfloat16)

    in_maps = []
    for b in range(B):
        maskf = np.ascontiguousarray(
            mask[b].astype(np.float32).reshape(NCH, 128).T * -1.0e9
        )
        in_maps.append(
            {
                "x": np.ascontiguousarray(x[b]).astype(ml_dtypes.bfloat16),
                "maskf": maskf,
                "wq": wq,
                "wk": wk,
                "wo": wo,
                "bqr": bqr,
                "bkr": bkr,
                "bo_bc": bo_bc,
                "ident": ident,
                "identb": identb,
            }
        )
    return in_maps


def _get_nc(repeat=1):
    key = ("nc", repeat)
    if key not in _cache:
        _cache[key] = _build_nc(repeat=repeat)
    return _cache[key]


def kernel(**inputs):
    from concourse.bass_utils import run_bass_kernel_spmd

    nc = _get_nc()
    in_maps = _host_inputs(inputs)
    res = run_bass_kernel_spmd(nc, in_maps, core_ids=list(range(B)))
    out = np.stack([res.results[b]["out"] for b in range(B)], axis=0)
    return out.astype(np.float32, copy=False)


def _get_runner(repeat=1):
    """Cached jitted SPMD executable (mirrors bass2jax.run_bass_via_pjrt) so
    repeat executions skip retrace/recompile — used for timing."""
    key = ("runner", repeat)
    if key in _cache:
        return _cache[key]
    import jax
    from jax.sharding import Mesh, PartitionSpec
    from jax.experimental.shard_map import shard_map
    from concourse import mybir
    from concourse import bass2jax

    nc = _get_nc(repeat=repeat)
    bass2jax.install_neuronx_cc_hook()
    part_name = nc.partition_id_tensor.name if nc.partition_id_tensor else None
    in_names, out_names, out_avals, zero_outs = [], [], [], []
    for alloc in nc.m.functions[0].allocations:
        if not isinstance(alloc, mybir.MemoryLocationSet):
            continue
        name = alloc.memorylocations[0].name
        if alloc.kind == "ExternalInput":
            if name != part_name:
                in_names.append(name)
        elif alloc.kind == "ExternalOutput":
            out_names.append(name)
            shape = tuple(alloc.tensor_shape)
            dtype = mybir.dt.np(alloc.dtype)
            out_avals.append(jax.core.ShapedArray(shape, dtype))
            zero_outs.append(np.zeros(shape, dtype))
    n_params = len(in_names)
    all_in_names = in_names + out_names
    if part_name is not None:
        all_in_names = all_in_names + [part_name]

    def _body(*args):
        operands = list(args)
        if part_name is not None:
            operands.append(bass2jax.partition_id_tensor())
        outs = bass2jax._bass_exec_p.bind(
            *operands,
            out_avals=tuple(out_avals),
            in_names=tuple(all_in_names),
            out_names=tuple(out_names),
            lowering_input_output_aliases=(),
            sim_require_finite=True,
            sim_require_nnan=True,
            nc=nc,
        )
        return tuple(outs)

    devices = jax.devices()[:B]
    mesh = Mesh(np.asarray(devices), ("core",))
    n_outs = len(out_names)
    sharded = jax.jit(
        shard_map(
            _body,
            mesh=mesh,
            in_specs=(PartitionSpec("core"),) * (n_params + n_outs),
            out_specs=(PartitionSpec("core"),) * n_outs,
            check_rep=False,
        ),
        keep_unused=True,
    )
    _cache[key] = (sharded, in_names, out_names, zero_outs, mesh)
    return _cache[key]


def _prepared_args(inputs, repeat=1):
    """Device-resident args for the timing runner. Cached per repeat level —
    re-uploading 170+ MB per measurement block degrades the tunnel and
    drifts the fixed overhead between paired measurements."""
    import jax
    from jax.sharding import NamedSharding, PartitionSpec

    key = ("args", repeat)
    if key in _cache:
        return _cache[key]
    sharded, in_names, out_names, zero_outs, mesh = _get_runner(repeat=repeat)
    in_maps = _host_inputs(inputs)
    concat_in = [
        np.concatenate([np.asarray(in_maps[c][n]) for c in range(B)], axis=0)
        for n in in_names
    ]
    concat_zeros = [
        np.zeros((B * z.shape[0], *z.shape[1:]), z.dtype) for z in zero_outs
    ]
    sh = NamedSharding(mesh, PartitionSpec("core"))
    args = [jax.device_put(a, sh) for a in concat_in + concat_zeros]
    jax.block_until_ready(args)
    _cache[key] = (sharded, args)
    return sharded, args


def steady_rate(inputs, repeat=1, n_iter=256):
    """Steady-state pipelined wall time per dispatch, seconds/call."""
    import jax, time

    sharded, args = _prepared_args(inputs, repeat=repeat)
    out = sharded(*args)
    jax.block_until_ready(out)
    t0 = time.time()
    out = None
    for _ in range(n_iter):
        out = sharded(*args)
    jax.block_until_ready(out)
    return (time.time() - t0) / n_iter


def timed_run(inputs, n_iter=256):
    """Amortized wall time per kernel execution in ns (pipelined dispatch;
    includes the fixed per-call axon-tunnel overhead)."""
    return steady_rate(inputs, repeat=1, n_iter=n_iter) * 1e9


# revision 38
# speedup vs baseline: 1.7359x; 1.7359x over previous
#!/usr/bin/env python
"""Multi-head attention (nn_MultiHeadAttention) Trainium2 Bass kernel, v2.

Problem: B=8, S=1024, n_hidden=1024, 16 heads x 64 dim. V projection == K
projection (reference quirk). Output = softmax(mask + QK^T/8) @ K @ Wo + bo.

Strategy: batch-parallel across the 8 NeuronCores (core b handles batch b,
weights replicated, zero collectives). Per core, a software-pipelined loop
over the 8 hidden tiles t (= head pairs 2t, 2t+1) keeps PE, ACT and DVE all
busy:

  iteration t emits, interleaved per key-chunk kc:
    logits^T(t, kc)   [128k, 1024q] = (K^T_t)^T-contract Q^T_t   (PE, fp32r)
    E(t, kc)          = exp(logits^T/8 + mask_bias) -> bf16       (ACT)
    att(t-1, qc=kc)   [128q, 2x65]  = E^T-contract V_aug (bf16 PE; column
                      64 of each head block = softmax denominator via the
                      ones column carried in V)
    normalize         DVE reciprocal + per-partition tensor_scalar_mul
    att^T(t-1)        PE transposes of the normalized [128q, 128d] blocks
                      (head pair packed on partitions) -> attT tile layout
    proj(t+1)         Q^T/K^T m-tile projections + V transposes (PE + DVE)

  epilogue: out[q, m] = attT^T-contract Wo (bf16) + bo  (direct DRAM layout)

The softmax skips the max-subtraction: logits are O(6), exp stays in fp32
range, masked keys produce exp(-1e9) == 0 exactly. Scores/V/att/Wo run in
bf16 (errors ~0.4%, far inside the 2e-2 gate); the x/Wq/Wk/logits path stays
fp32r.
"""
import sys
import os

sys.path.insert(0, "/opt/trn_rl_repo")
os.environ.setdefault("JAX_COMPILATION_CACHE_DIR", "/tmp/jax_comp_cache")

import numpy as np

B, S, H, NH, DH = 8, 1024, 1024, 16, 64
NT = H // 128   # 8 partition tiles of hidden (= head pairs)
NCH = S // 128  # 8 key chunks
NQ = S // 512   # 2 query 512-tiles
DV = DH + 1     # V block width (ones column at 64)

_cache = {}


def _build_nc(repeat=1):
    import concourse.bacc as bacc
    import concourse.tile as tile
    from concourse import mybir
    from contextlib import ExitStack

    F32 = mybir.dt.float32
    F32R = mybir.dt.float32r
    BF16 = mybir.dt.bfloat16

    nc = bacc.Bacc("TRN2", target_bir_lowering=False, debug=False)

    x_d = nc.dram_tensor("x", [S, H], BF16, kind="ExternalInput").ap()
    maskf_d = nc.dram_tensor("maskf", [128, NCH], F32, kind="ExternalInput").ap()
    wq_d = nc.dram_tensor("wq", [H, H], BF16, kind="ExternalInput").ap()  # pre-tiled [m*128+p, k*128+mm]
    wk_d = nc.dram_tensor("wk", [H, H], BF16, kind="ExternalInput").ap()  # pre-tiled
    wo_d = nc.dram_tensor("wo", [H, H], BF16, kind="ExternalInput").ap()
    bqr_d = nc.dram_tensor("bqr", [128, NT], F32, kind="ExternalInput").ap()
    bkr_d = nc.dram_tensor("bkr", [128, NT], F32, kind="ExternalInput").ap()
    bo_d = nc.dram_tensor("bo_bc", [128, H], F32, kind="ExternalInput").ap()
    idb_d = nc.dram_tensor("identb", [128, 128], BF16, kind="ExternalInput").ap()
    out_d = nc.dram_tensor("out", [S, H], F32, kind="ExternalOutput").ap()

    with tile.TileContext(nc) as tc, ExitStack() as top:
        pools = {
            "misc": top.enter_context(tc.tile_pool(name="misc", bufs=1)),
            "xT": top.enter_context(tc.tile_pool(name="xT", bufs=1)),
            "xs": top.enter_context(tc.tile_pool(name="xs", bufs=1)),
            "wst": top.enter_context(tc.tile_pool(name="wst", bufs=4)),
            "QT": top.enter_context(tc.tile_pool(name="QTp", bufs=3)),
            "KT": top.enter_context(tc.tile_pool(name="KTp", bufs=3)),
            "V": top.enter_context(tc.tile_pool(name="Vp", bufs=1)),
            "E": top.enter_context(tc.tile_pool(name="Ep", bufs=32)),
            "attT": top.enter_context(tc.tile_pool(name="attTp", bufs=1)),
            "asb": top.enter_context(tc.tile_pool(name="asbp", bufs=2)),
            "rc": top.enter_context(tc.tile_pool(name="rcp", bufs=2)),
            "wo": top.enter_context(tc.tile_pool(name="wo", bufs=1)),
            "pp": top.enter_context(tc.tile_pool(name="pp", bufs=2, space="PSUM")),
            "lg": top.enter_context(tc.tile_pool(name="lg", bufs=2, space="PSUM")),
            "att": top.enter_context(tc.tile_pool(name="att", bufs=2, space="PSUM")),
        }
        misc = pools["misc"]
        maskf = misc.tile([128, NCH], F32)
        bqr = misc.tile([128, NT], F32)
        bkr = misc.tile([128, NT], F32)
        bo_bc = misc.tile([128, H], F32)
        identb = misc.tile([128, 128], BF16)

        for _rep in range(repeat):
            _emit_body(
                nc, tc, tile, mybir, pools,
                x_d, wq_d, wk_d, wo_d, out_d,
                maskf, bqr, bkr, bo_bc, identb,
                maskf_d, bqr_d, bkr_d, bo_d, idb_d,
                first=(_rep == 0),
            )

    nc.compile()
    return nc


def _emit_body(nc, tc, tile, mybir, pools,
               x_d, wq_d, wk_d, wo_d, out_d,
               maskf, bqr, bkr, bo_bc, identb,
               maskf_d, bqr_d, bkr_d, bo_d, idb_d, first=True):
    F32 = mybir.dt.float32
    BF16 = mybir.dt.bfloat16
    AF = mybir.ActivationFunctionType

    if True:
        wst_p = pools["wst"]
        QT_p, KT_p = pools["QT"], pools["KT"]
        E_p, asb_p, rc_p = pools["E"], pools["asb"], pools["rc"]
        pp_pool, lg_pool, att_pool = pools["pp"], pools["lg"], pools["att"]
        xT = pools["xT"].tile([128, NT * S], BF16, tag="xT", name="xT")

        Wq_sb = {}    # t -> staged weight tile
        Wk_sb = {}

        def dma_w(t):
            Wq_sb[t] = wst_p.tile([128, H], BF16, tag="w", name=f"wq_{t}")
            Wk_sb[t] = wst_p.tile([128, H], BF16, tag="w", name=f"wk_{t}")
            nc.sync.dma_start(Wq_sb[t][:], wq_d[t * 128 : (t + 1) * 128, :])
            nc.sync.dma_start(Wk_sb[t][:], wk_d[t * 128 : (t + 1) * 128, :])

        # ---- Phase A: load x, transpose to x^T ---------------------------
        # x chunks alternate between the SP and ACT HWDGE queues so two DMA
        # engines stream in parallel. All pools are shared across repeats, so
        # in the repeated (timing) NEFF this phase overlaps the previous
        # repeat's output projection instead of waiting on a drain barrier.
        xs = pools["xs"].tile([128, NCH * H], BF16, tag="xs", name="xs")
        for sc in range(NCH):
            eng = nc.sync if sc % 2 == 0 else nc.scalar
            eng.dma_start(
                xs[:, sc * H : (sc + 1) * H],
                x_d[sc * 128 : (sc + 1) * 128, :],
            )
        dma_w(0)
        if first:
            nc.scalar.dma_start(maskf[:], maskf_d)
            nc.scalar.dma_start(identb[:], idb_d)
        dma_w(1)
        if first:
            nc.sync.dma_start(bqr[:], bqr_d)
            nc.sync.dma_start(bkr[:], bkr_d)
            nc.sync.dma_start(bo_bc[:], bo_d)
        for g in range(2):
            for hc in range(NT):
                pt = pp_pool.tile([128, 512], BF16, tag="pp", name="pt")
                for j in range(4):
                    sc = g * 4 + j
                    nc.tensor.transpose(
                        pt[:, 128 * j : 128 * (j + 1)],
                        xs[:, sc * H + hc * 128 : sc * H + (hc + 1) * 128],
                        identb[:],
                    )
                xt_dst = xT[:, hc * S + g * 512 : hc * S + (g + 1) * 512]
                if hc % 2 == 0:
                    nc.vector.tensor_copy(xt_dst, pt[:])
                else:
                    nc.scalar.activation(xt_dst, pt[:], AF.Identity, bias=0.0)

        V = pools["V"].tile([128, NH * NCH * DV], BF16, tag="V", name="V")
        attT = pools["attT"].tile([128, NT * S], BF16, tag="attT", name="attT")
        V_blocks = V[:].rearrange("p (g o) -> p g o", o=DV)
        nc.vector.memset(V_blocks[:, :, DH : DH + 1], 1.0)

        QT = {}       # t -> [128, S] tile (head pair 2t,2t+1 on partitions)
        KT = {}
        E_t = {}      # (t, h2, kc) -> E tile
        asb = {}      # t -> normalized att sbuf tile [128, S] bf16
        rct = {}      # t -> reciprocal tile [128, 16]

        def proj_piece(t, piece, pp_p):
            # pieces 0..3: Q/K projections by 512-chunk; 4,5: V transposes
            if piece < 4:
                is_q = piece < 2
                n = piece % 2
                w_m = Wq_sb[t] if is_q else Wk_sb[t]
                brow = bqr if is_q else bkr
                dct, pool, tg = (QT, QT_p, "qt") if is_q else (KT, KT_p, "kt")
                if n == 0:
                    dct[t] = pool.tile([128, S], BF16, tag=tg, name=f"{tg}_{t}")
                dst = dct[t]
                pp = pp_p.tile([128, 512], F32, tag="pp", name="pp")
                for k in range(NT):
                    nc.tensor.matmul(
                        pp[:],
                        w_m[:, k * 128 : (k + 1) * 128],
                        xT[:, k * S + n * 512 : k * S + (n + 1) * 512],
                        start=(k == 0),
                        stop=(k == NT - 1),
                    )
                nc.vector.tensor_scalar_add(
                    dst[:, n * 512 : (n + 1) * 512], pp[:], brow[:, t : t + 1]
                )
            else:
                h2 = piece - 4
                h = 2 * t + h2
                pv = pp_p.tile([128, 512], BF16, tag="pp", name="pv")
                for c in range(NCH):
                    nc.tensor.transpose(
                        pv[:, c * DH : (c + 1) * DH],
                        KT[t][64 * h2 : 64 * h2 + 64, c * 128 : (c + 1) * 128],
                        identb[64 * h2 : 64 * h2 + 64, 64 * h2 : 64 * h2 + 64],
                    )
                nc.vector.tensor_copy(
                    V_blocks[:, h * NCH : (h + 1) * NCH, 0:DH],
                    pv[:].rearrange("p (c d) -> p c d", d=DH),
                )

        def lg_exp(t, kc, lg_p):
            for h2 in range(2):
                lg = lg_p.tile([128, S], F32, tag="lg", name="lg")
                for n in range(NQ):
                    nc.tensor.matmul(
                        lg[:, n * 512 : (n + 1) * 512],
                        KT[t][64 * h2 : 64 * h2 + 64, kc * 128 : (kc + 1) * 128],
                        QT[t][64 * h2 : 64 * h2 + 64, n * 512 : (n + 1) * 512],
                        start=True,
                        stop=True,
                    )
                E = E_p.tile([128, S], BF16, tag="E", name="E")
                nc.scalar.activation(
                    E[:], lg[:], AF.Exp, bias=maskf[:, kc : kc + 1], scale=0.125
                )
                E_t[(t, h2, kc)] = E

        def att_qc(t, qc, att_p, lg_p=None):
            if qc == 0:
                asb[t] = asb_p.tile([128, S], BF16, tag="asb", name=f"asb_{t}")
                rct[t] = rc_p.tile([128, 16], F32, tag="rc", name=f"rc_{t}")
            if lg_p is not None and qc % 2 == 1:
                # final tile: lg pool is idle, borrow it for ring depth 4
                attb = lg_p.tile([128, 2 * DV], F32, tag="lg", name="attb")
            else:
                attb = att_p.tile([128, 2 * DV], F32, tag="att", name="attb")
            for h2 in range(2):
                h = 2 * t + h2
                for kc in range(NCH):
                    nc.tensor.matmul(
                        attb[:, DV * h2 : DV * h2 + DV],
                        E_t[(t, h2, kc)][:, qc * 128 : (qc + 1) * 128],
                        V[:, (h * NCH + kc) * DV : (h * NCH + kc + 1) * DV],
                        start=(kc == 0),
                        stop=(kc == NCH - 1),
                    )
            for h2 in range(2):
                nc.vector.reciprocal(
                    rct[t][:, 2 * qc + h2 : 2 * qc + h2 + 1],
                    attb[:, DV * h2 + DH : DV * h2 + DH + 1],
                )
            for h2 in range(2):
                nc.vector.tensor_scalar_mul(
                    asb[t][:, qc * 128 + 64 * h2 : qc * 128 + 64 * h2 + 64],
                    attb[:, DV * h2 : DV * h2 + DH],
                    rct[t][:, 2 * qc + h2 : 2 * qc + h2 + 1],
                )

        def attT_tp(t, g, pp_p):
            pt = pp_p.tile([128, 512], BF16, tag="pp", name="ptt")
            for j in range(4):
                qc = 4 * g + j
                nc.tensor.transpose(
                    pt[:, j * 128 : (j + 1) * 128],
                    asb[t][:, qc * 128 : (qc + 1) * 128],
                    identb[:],
                )
            nc.vector.tensor_copy(
                attT[:, t * S + g * 512 : t * S + (g + 1) * 512], pt[:]
            )

        # ---- main software-pipelined loop --------------------------------
        wo_sb = None
        wo_p = pools["wo"]
        if True:
            pp_p, lg_p, att_p = pp_pool, lg_pool, att_pool
            if True:
                for piece in range(6):
                    proj_piece(0, piece, pp_p)
                for t in range(NT + 1):
                    for kc in range(NCH):
                        if t < NT:
                            lg_exp(t, kc, lg_p)
                        if t >= 1:
                            att_qc(t - 1, kc, att_p, lg_p if t == NT else None)
                            if kc == 3:
                                attT_tp(t - 1, 0, pp_p)
                            elif kc == 7:
                                attT_tp(t - 1, 1, pp_p)
                        if t + 1 < NT and 1 <= kc <= 6:
                            if kc == 1 and t + 2 < NT:
                                dma_w(t + 2)
                            proj_piece(t + 1, kc - 1, pp_p)
                        if t == NT - 1 and kc == 0:
                            # prefetch Wo while the last attention tiles run
                            wo_sb = wo_p.tile([128, NT * H], BF16)
                            for c in range(NT):
                                nc.sync.dma_start(
                                    wo_sb[:, c * H : (c + 1) * H],
                                    wo_d[c * 128 : (c + 1) * 128, :],
                                )
                    if t >= 1:
                        for h2 in range(2):
                            for kc in range(NCH):
                                E_t.pop((t - 1, h2, kc), None)

                # ---- Phase E: output projection. Reuses the pp/asb rings
                # (no new pools -> no drain barrier before the first matmul).
                for qt in range(NT):
                    for mt in range(NQ):
                        po = pp_p.tile([128, 512], F32, tag="pp", name="po")
                        for c in range(NT):
                            nc.tensor.matmul(
                                po[:],
                                attT[:, c * S + qt * 128 : c * S + (qt + 1) * 128],
                                wo_sb[:, c * H + mt * 512 : c * H + (mt + 1) * 512],
                                start=(c == 0),
                                stop=(c == NT - 1),
                            )
                        # weight-staging ring is retired by now; borrow it so
                        # four out-tiles can be in flight over the DMA queues
                        ob = wst_p.tile([128, 512], F32, tag="w", name="ob")
                        nc.vector.tensor_add(
                            ob[:], po[:], bo_bc[:, mt * 512 : (mt + 1) * 512]
                        )
                        eng = nc.sync if (qt + mt) % 2 == 0 else nc.scalar
                        eng.dma_start(
                            out_d[qt * 128 : (qt + 1) * 128, mt * 512 : (mt + 1) * 512],
                            ob[:],
                        )


def _host_inputs(inputs):
    """Host-side prep: per-core input dicts (core b <- batch b)."""
    import ml_dtypes

    x = np.asarray(inputs["x"], dtype=np.float32)
    mask = np.asarray(inputs["padding_mask"])

    def _pretile(w):
        # w[k*128+p, m*128+mm] -> out[m*128+p, k*128+mm]
        w = np.asarray(w, dtype=np.float32).reshape(NT, 128, NT, 128)
        return np.ascontiguousarray(w.transpose(2, 1, 0, 3).reshape(H, H))

    wq = _pretile(inputs["Wq"]).astype(ml_dtypes.bfloat16)
    wk = _pretile(inputs["Wk"]).astype(ml_dtypes.bfloat16)
    wo = np.ascontiguousarray(
        np.asarray(inputs["Wo"], dtype=np.float32).astype(ml_dtypes.bfloat16)
    )
    bq = np.asarray(inputs["bq"], dtype=np.float32)
    bk = np.asarray(inputs["bk"], dtype=np.float32)
    bo = np.asarray(inputs["bo"], dtype=np.float32)

    bqr = np.ascontiguousarray(bq.reshape(NT, 128).T)
    bkr = np.ascontiguousarray(bk.reshape(NT, 128).T)
    bo_bc = np.ascontiguousarray(np.tile(bo[None, :], (128, 1)))
    identb = np.eye(128, dtype=np.float32).astype(ml_dtypes.bfloat16)

    in_maps = []
    for b in range(B):
        maskf = np.ascontiguousarray(
            mask[b].astype(np.float32).reshape(NCH, 128).T * -1.0e9
        )
        in_maps.append(
            {
                "x": np.ascontiguousarray(x[b]).astype(ml_dtypes.bfloat16),
                "maskf": maskf,
                "wq": wq,
                "wk": wk,
                "wo": wo,
                "bqr": bqr,
                "bkr": bkr,
                "bo_bc": bo_bc,
                "identb": identb,
            }
        )
    return in_maps


def _get_nc(repeat=1):
    key = ("nc", repeat)
    if key not in _cache:
        _cache[key] = _build_nc(repeat=repeat)
    return _cache[key]


def kernel(**inputs):
    from concourse.bass_utils import run_bass_kernel_spmd

    nc = _get_nc()
    in_maps = _host_inputs(inputs)
    res = run_bass_kernel_spmd(nc, in_maps, core_ids=list(range(B)))
    out = np.stack([res.results[b]["out"] for b in range(B)], axis=0)
    return out.astype(np.float32, copy=False)


def _get_runner(repeat=1):
    """Cached jitted SPMD executable (mirrors bass2jax.run_bass_via_pjrt) so
    repeat executions skip retrace/recompile — used for timing."""
    key = ("runner", repeat)
    if key in _cache:
        return _cache[key]
    import jax
    from jax.sharding import Mesh, PartitionSpec
    from jax.experimental.shard_map import shard_map
    from concourse import mybir
    from concourse import bass2jax

    nc = _get_nc(repeat=repeat)
    bass2jax.install_neuronx_cc_hook()
    part_name = nc.partition_id_tensor.name if nc.partition_id_tensor else None
    in_names, out_names, out_avals, zero_outs = [], [], [], []
    for alloc in nc.m.functions[0].allocations:
        if not isinstance(alloc, mybir.MemoryLocationSet):
            continue
        name = alloc.memorylocations[0].name
        if alloc.kind == "ExternalInput":
            if name != part_name:
                in_names.append(name)
        elif alloc.kind == "ExternalOutput":
            out_names.append(name)
            shape = tuple(alloc.tensor_shape)
            dtype = mybir.dt.np(alloc.dtype)
            out_avals.append(jax.core.ShapedArray(shape, dtype))
            zero_outs.append(np.zeros(shape, dtype))
    n_params = len(in_names)
    all_in_names = in_names + out_names
    if part_name is not None:
        all_in_names = all_in_names + [part_name]

    def _body(*args):
        operands = list(args)
        if part_name is not None:
            operands.append(bass2jax.partition_id_tensor())
        outs = bass2jax._bass_exec_p.bind(
            *operands,
            out_avals=tuple(out_avals),
            in_names=tuple(all_in_names),
            out_names=tuple(out_names),
            lowering_input_output_aliases=(),
            sim_require_finite=True,
            sim_require_nnan=True,
            nc=nc,
        )
        return tuple(outs)

    devices = jax.devices()[:B]
    mesh = Mesh(np.asarray(devices), ("core",))
    n_outs = len(out_names)
    sharded = jax.jit(
        shard_map(
            _body,
            mesh=mesh,
            in_specs=(PartitionSpec("core"),) * (n_params + n_outs),
            out_specs=(PartitionSpec("core"),) * n_outs,
            check_rep=False,
        ),
        keep_unused=True,
    )
    _cache[key] = (sharded, in_names, out_names, zero_outs, mesh)
    return _cache[key]


def _prepared_args(inputs, repeat=1):
    """Device-resident args for the timing runner. Cached per repeat level —
    re-uploading 170+ MB per measurement block degrades the tunnel and
    drifts the fixed overhead between paired measurements."""
    import jax
    from jax.sharding import NamedSharding, PartitionSpec

    key = ("args", repeat)
    if key in _cache:
        return _cache[key]
    sharded, in_names, out_names, zero_outs, mesh = _get_runner(repeat=repeat)
    in_maps = _host_inputs(inputs)
    concat_in = [
        np.concatenate([np.asarray(in_maps[c][n]) for c in range(B)], axis=0)
        for n in in_names
    ]
    concat_zeros = [
        np.zeros((B * z.shape[0], *z.shape[1:]), z.dtype) for z in zero_outs
    ]
    sh = NamedSharding(mesh, PartitionSpec("core"))
    args = [jax.device_put(a, sh) for a in concat_in + concat_zeros]
    jax.block_until_ready(args)
    _cache[key] = (sharded, args)
    return sharded, args


def steady_rate(inputs, repeat=1, n_iter=256):
    """Steady-state pipelined wall time per dispatch, seconds/call."""
    import jax, time

    sharded, args = _prepared_args(inputs, repeat=repeat)
    out = sharded(*args)
    jax.block_until_ready(out)
    t0 = time.time()
    out = None
    for _ in range(n_iter):
        out = sharded(*args)
    jax.block_until_ready(out)
    return (time.time() - t0) / n_iter


def timed_run(inputs, n_iter=256):
    """Amortized wall time per kernel execution in ns (pipelined dispatch;
    includes the fixed per-call axon-tunnel overhead)."""
    return steady_rate(inputs, repeat=1, n_iter=n_iter) * 1e9
